# revision 18
# baseline (speedup 1.0000x reference)
"""GraphSAGE-style encoder kernel for Trainium2 (Bass/Tile), 8-core SPMD.

Computation (see reference):
    agg = features[neigh_idx].mean(axis=1)          # [B, F]
    y   = weight @ agg.T                            # [E, B]
    out = where(y >= 0, y, y * 11/48)               # RReLU eval mode

Sharding: data-parallel over the batch. Core c handles neigh_idx rows
[c*2048, (c+1)*2048); features + weight are replicated. Each core:
  - indirect-DMA-gathers its 20480 neighbor rows (2KB each) from HBM,
    128 nodes x 10 neighbors per gather instruction,
  - tree-sums the 10 rows per node on DVE,
  - transposes agg tiles on the tensor engine (f onto partitions),
  - matmuls with (weight/10)^T accumulated over 4 f-chunks into PSUM,
  - applies leaky-relu as max(y, slope*y) and DMAs out [128e, 512b] tiles.
The per-core 42 MB gather is the roofline term (memory regime).
"""

import numpy as np

N_CORES = 8
N_NODES, FEAT = 100000, 512
EMBED = 256
BATCH, S = 16384, 10
B_CORE = BATCH // N_CORES          # 2048 batch rows per core
P = 128                            # partitions
TILES = B_CORE // P                # 16 node-tiles per core
TPB = 4                            # node-tiles per output block
BLOCKS = TILES // TPB              # 4 blocks -> psum free dim 512
Q = TPB * P                        # 512 nodes per block
RRELU_SLOPE = (1.0 / 8.0 + 1.0 / 3.0) / 2.0   # 11/48

_CACHE = {}


def build_program():
    import concourse.bass as bass
    import concourse.bacc as bacc
    import concourse.mybir as mybir
    import concourse.tile as tile
    from concourse.masks import make_identity

    f32 = mybir.dt.float32
    i32 = mybir.dt.int32

    # Bacc (not plain Bass): its compile() runs the wait-legalization passes
    # (move_matmul_waits_to_ldweights / generate_event_semaphores) that
    # enforce the 1-wait-per-instruction HW constraint walrus checks.
    nc = bacc.Bacc(trn_type="TRN2", num_devices=N_CORES, enable_asserts=False)

    feat = nc.dram_tensor("features", [N_NODES, FEAT], f32, kind="ExternalInput").ap()
    wts = nc.dram_tensor("weight", [EMBED, FEAT], f32, kind="ExternalInput").ap()
    # int64 neigh_idx rows viewed as int32 pairs (little-endian: even cols
    # hold the values, odd cols are zero).
    idx = nc.dram_tensor("idx32", [B_CORE, 2 * S], i32, kind="ExternalInput").ap()
    out = nc.dram_tensor("out", [EMBED, B_CORE], f32, kind="ExternalOutput").ap()

    with tile.TileContext(nc) as tc:
        with (
            tc.tile_pool(name="const", bufs=1) as const,
            tc.tile_pool(name="xp", bufs=3) as xp,
            tc.tile_pool(name="sump", bufs=2) as sump,
            tc.tile_pool(name="aggp", bufs=2) as aggp,
            tc.tile_pool(name="aggTp", bufs=2) as aggTp,
            tc.tile_pool(name="outp", bufs=3) as outp,
            tc.tile_pool(name="psT", bufs=2, space="PSUM") as psT,
            tc.tile_pool(name="psO", bufs=4, space="PSUM") as psO,
        ):
            # Preload + compact ALL neighbor offsets once, FIRST (the first
            # gather is gated on this; W loads go to the scalar HWDGE queue
            # so they don't delay it). Gathers then only depend on this
            # constant tile, never on a per-tile DVE compact racing the
            # summation work.
            idx_all = const.tile([P, TILES * 2 * S], i32)
            nc.sync.dma_start(
                out=idx_all[:].rearrange("p (g j) -> p g j", j=2 * S),
                in_=idx[:].rearrange("(g p) j -> p g j", p=P),
            )
            off_all = const.tile([P, TILES * S], i32)
            nc.vector.tensor_copy(
                off_all[:].rearrange("p (g s) -> p g s", s=S),
                idx_all[:].rearrange("p (g s two) -> p g s two", two=2, s=S)[
                    :, :, :, 0
                ],
            )

            ident = const.tile([P, P], f32)
            make_identity(nc, ident[:])

            # --- one-time weight prep: scale by 1/10 (folds the neighbor
            # mean; leaky-relu is positively homogeneous) and transpose to
            # [f, e] chunks for the matmul lhsT.
            w_sb = const.tile([P, 2 * FEAT], f32)      # cols [h*FEAT..] = W rows h*128..
            nc.scalar.dma_start(out=w_sb[:, :FEAT], in_=wts[0:P, :])
            nc.scalar.dma_start(out=w_sb[:, FEAT:], in_=wts[P : 2 * P, :])
            ws_sb = const.tile([P, 2 * FEAT], f32)
            # one scale op per W-half: walrus allows a single sync wait on
            # TensorScalar instructions, and the two loads complete on
            # different DMA semaphore lanes.
            nc.vector.tensor_scalar_mul(ws_sb[:, :FEAT], w_sb[:, :FEAT], 1.0 / S)
            nc.vector.tensor_scalar_mul(ws_sb[:, FEAT:], w_sb[:, FEAT:], 1.0 / S)

            wt_sb = [
                const.tile([P, 2 * P], f32, name=f"wt{c}", tag=f"wt{c}")
                for c in range(4)
            ]
            for h in range(2):
                wt_ps = psT.tile([P, 4 * P], f32)
                for c in range(4):
                    nc.tensor.transpose(
                        out=wt_ps[:, c * P : (c + 1) * P],
                        in_=ws_sb[:, h * FEAT + c * P : h * FEAT + (c + 1) * P],
                        identity=ident[:],
                    )
                for c in range(4):
                    nc.vector.tensor_copy(
                        wt_sb[c][:, h * P : (h + 1) * P],
                        wt_ps[:, c * P : (c + 1) * P],
                    )

            # --- main loop
            for blk in range(BLOCKS):
                aggT = aggTp.tile([P, 4 * Q], f32)   # [f-part, (chunk c)*(Q nodes)]
                for t in range(TPB):
                    g = blk * TPB + t
                    # One indirect DMA per neighbor column: the HW DGE pairs
                    # ONE offset with each per-partition descriptor (walrus
                    # indirect loads are 2-D only), so a [128, 512] dest with
                    # [128, 1] offsets is the unit of gather. (An inline
                    # CCE-add variant was measured slower: the accumulate
                    # path raises the per-instruction DGE cost ~35%.)
                    x_t = xp.tile([P, S * FEAT], f32)
                    for j in range(S):
                        nc.gpsimd.indirect_dma_start(
                            out=x_t[:, j * FEAT : (j + 1) * FEAT],
                            out_offset=None,
                            in_=feat[:],
                            in_offset=bass.IndirectOffsetOnAxis(
                                ap=off_all[:, g * S + j : g * S + j + 1], axis=0
                            ),
                        )

                    # tree-sum the 10 rows per node: 10 -> 5 -> (4->2->1) + leftover
                    y5 = sump.tile([P, 5 * FEAT], f32)
                    xv = x_t[:].rearrange("p (s two f) -> p s two f", two=2, f=FEAT)
                    nc.vector.tensor_add(
                        y5[:].rearrange("p (s f) -> p s f", f=FEAT),
                        xv[:, :, 0, :],
                        xv[:, :, 1, :],
                    )
                    y2 = sump.tile([P, 2 * FEAT], f32)
                    y4 = y5[:, 0 : 4 * FEAT].rearrange(
                        "p (s two f) -> p s two f", two=2, f=FEAT
                    )
                    nc.vector.tensor_add(
                        y2[:].rearrange("p (s f) -> p s f", f=FEAT),
                        y4[:, :, 0, :],
                        y4[:, :, 1, :],
                    )
                    agg = aggp.tile([P, FEAT], f32)
                    nc.vector.tensor_add(agg[:], y2[:, :FEAT], y2[:, FEAT:])
                    nc.vector.tensor_add(
                        agg[:], agg[:], y5[:, 4 * FEAT : 5 * FEAT]
                    )

                    # transpose agg -> aggT columns (f onto partitions)
                    ps_t = psT.tile([P, 4 * P], f32)
                    for c in range(4):
                        nc.tensor.transpose(
                            out=ps_t[:, c * P : (c + 1) * P],
                            in_=agg[:, c * P : (c + 1) * P],
                            identity=ident[:],
                        )
                    # PSUM -> aggT copy on the (otherwise idle) scalar engine
                    # to keep DVE turnaround fast for the x_t release chain.
                    nc.scalar.activation(
                        aggT[:].rearrange("p (c n) -> p c n", n=Q)[
                            :, :, t * P : (t + 1) * P
                        ],
                        ps_t[:].rearrange("p (c n) -> p c n", n=P),
                        mybir.ActivationFunctionType.Copy,
                    )

                # matmul + leaky relu + store, per embed-half
                for h in range(2):
                    ps_o = psO.tile([P, Q], f32)
                    for c in range(4):
                        nc.tensor.matmul(
                            out=ps_o[:],
                            lhsT=wt_sb[c][:, h * P : (h + 1) * P],
                            rhs=aggT[:, c * Q : (c + 1) * Q],
                            start=(c == 0),
                            stop=(c == 3),
                        )
                    tmp = outp.tile([P, Q], f32)
                    o_sb = outp.tile([P, Q], f32)
                    nc.vector.tensor_scalar_mul(tmp[:], ps_o[:], RRELU_SLOPE)
                    nc.vector.tensor_max(o_sb[:], ps_o[:], tmp[:])
                    nc.sync.dma_start(
                        out=out[h * P : (h + 1) * P, blk * Q : (blk + 1) * Q],
                        in_=o_sb[:],
                    )
    nc.compile()
    return nc


def _prep_inputs(features, weight, neigh_idx):
    feats = np.ascontiguousarray(np.asarray(features), dtype=np.float32)
    w = np.ascontiguousarray(np.asarray(weight), dtype=np.float32)
    idx = np.asarray(neigh_idx)
    assert feats.shape == (N_NODES, FEAT) and w.shape == (EMBED, FEAT)
    assert idx.shape == (BATCH, S)
    idx32 = (
        np.ascontiguousarray(idx.astype(np.int64))
        .view(np.int32)
        .reshape(BATCH, 2 * S)
    )
    return feats, w, idx32


def kernel(features, weight, neigh_idx):
    from concourse.bass_utils import run_bass_kernel_spmd

    if "nc" not in _CACHE:
        _CACHE["nc"] = build_program()
    nc = _CACHE["nc"]

    feats, w, idx32 = _prep_inputs(features, weight, neigh_idx)
    in_maps = [
        {
            "features": feats,
            "weight": w,
            "idx32": idx32[c * B_CORE : (c + 1) * B_CORE],
        }
        for c in range(N_CORES)
    ]
    res = run_bass_kernel_spmd(nc, in_maps, list(range(N_CORES)))
    return np.concatenate(
        [res.results[c]["out"] for c in range(N_CORES)], axis=1
    ).astype(np.float32)


# revision 23
# speedup vs baseline: 1.0131x; 1.0131x over previous
"""GraphSAGE-style encoder kernel for Trainium2 (Bass/Tile), 8-core SPMD.

Computation (see reference):
    agg = features[neigh_idx].mean(axis=1)          # [B, F]
    y   = weight @ agg.T                            # [E, B]
    out = where(y >= 0, y, y * 11/48)               # RReLU eval mode

Sharding: data-parallel over the batch. Core c handles neigh_idx rows
[c*2048, (c+1)*2048); features + weight are replicated. Each core:
  - indirect-DMA-gathers its 20480 neighbor rows (2KB each) from HBM,
    128 nodes x 10 neighbors per gather instruction,
  - tree-sums the 10 rows per node on DVE,
  - transposes agg tiles on the tensor engine (f onto partitions),
  - matmuls with (weight/10)^T accumulated over 4 f-chunks into PSUM,
  - applies leaky-relu as max(y, slope*y) and DMAs out [128e, 512b] tiles.
The per-core 42 MB gather is the roofline term (memory regime).
"""

import numpy as np

N_CORES = 8
N_NODES, FEAT = 100000, 512
EMBED = 256
BATCH, S = 16384, 10
B_CORE = BATCH // N_CORES          # 2048 batch rows per core
P = 128                            # partitions
TILES = B_CORE // P                # 16 node-tiles per core
TPB = 4                            # node-tiles per output block
BLOCKS = TILES // TPB              # 4 blocks -> psum free dim 512
Q = TPB * P                        # 512 nodes per block
RRELU_SLOPE = (1.0 / 8.0 + 1.0 / 3.0) / 2.0   # 11/48

_CACHE = {}


def build_program():
    import concourse.bass as bass
    import concourse.bacc as bacc
    import concourse.mybir as mybir
    import concourse.tile as tile
    from concourse.masks import make_identity

    f32 = mybir.dt.float32
    i32 = mybir.dt.int32

    # Bacc (not plain Bass): its compile() runs the wait-legalization passes
    # (move_matmul_waits_to_ldweights / generate_event_semaphores) that
    # enforce the 1-wait-per-instruction HW constraint walrus checks.
    nc = bacc.Bacc(trn_type="TRN2", num_devices=N_CORES, enable_asserts=False)

    feat = nc.dram_tensor("features", [N_NODES, FEAT], f32, kind="ExternalInput").ap()
    wts = nc.dram_tensor("weight", [EMBED, FEAT], f32, kind="ExternalInput").ap()
    # int64 neigh_idx rows viewed as int32 pairs (little-endian: even cols
    # hold the values, odd cols are zero).
    idx = nc.dram_tensor("idx32", [B_CORE, 2 * S], i32, kind="ExternalInput").ap()
    out = nc.dram_tensor("out", [EMBED, B_CORE], f32, kind="ExternalOutput").ap()

    with tile.TileContext(nc) as tc:
        with (
            tc.tile_pool(name="const", bufs=1) as const,
            tc.tile_pool(name="xp", bufs=3) as xp,
            tc.tile_pool(name="idxp", bufs=4) as idxp,
            tc.tile_pool(name="sump", bufs=2) as sump,
            tc.tile_pool(name="aggp", bufs=2) as aggp,
            tc.tile_pool(name="aggTp", bufs=2) as aggTp,
            tc.tile_pool(name="outp", bufs=3) as outp,
            tc.tile_pool(name="psT", bufs=2, space="PSUM") as psT,
            tc.tile_pool(name="psO", bufs=4, space="PSUM") as psO,
        ):
            ident = const.tile([P, P], f32)
            make_identity(nc, ident[:])

            # --- one-time weight prep: scale by 1/10 (folds the neighbor
            # mean; leaky-relu is positively homogeneous) and transpose to
            # [f, e] chunks for the matmul lhsT.
            w_sb = const.tile([P, 2 * FEAT], f32)      # cols [h*FEAT..] = W rows h*128..
            nc.sync.dma_start(out=w_sb[:, :FEAT], in_=wts[0:P, :])
            nc.sync.dma_start(out=w_sb[:, FEAT:], in_=wts[P : 2 * P, :])
            ws_sb = const.tile([P, 2 * FEAT], f32)
            # one scale op per W-half: walrus allows a single sync wait on
            # TensorScalar instructions, and the two loads complete on
            # different DMA semaphore lanes.
            nc.vector.tensor_scalar_mul(ws_sb[:, :FEAT], w_sb[:, :FEAT], 1.0 / S)
            nc.vector.tensor_scalar_mul(ws_sb[:, FEAT:], w_sb[:, FEAT:], 1.0 / S)

            wt_sb = [
                const.tile([P, 2 * P], f32, name=f"wt{c}", tag=f"wt{c}")
                for c in range(4)
            ]
            for h in range(2):
                wt_ps = psT.tile([P, 4 * P], f32)
                for c in range(4):
                    nc.tensor.transpose(
                        out=wt_ps[:, c * P : (c + 1) * P],
                        in_=ws_sb[:, h * FEAT + c * P : h * FEAT + (c + 1) * P],
                        identity=ident[:],
                    )
                for c in range(4):
                    nc.vector.tensor_copy(
                        wt_sb[c][:, h * P : (h + 1) * P],
                        wt_ps[:, c * P : (c + 1) * P],
                    )

            # --- main loop
            for blk in range(BLOCKS):
                aggT = aggTp.tile([P, 4 * Q], f32)   # [f-part, (chunk c)*(Q nodes)]
                for t in range(TPB):
                    g = blk * TPB + t
                    idx_t = idxp.tile([P, 2 * S], i32)
                    nc.sync.dma_start(
                        out=idx_t[:], in_=idx[g * P : (g + 1) * P, :]
                    )
                    off_t = idxp.tile([P, S], i32)
                    nc.vector.tensor_copy(
                        off_t[:],
                        idx_t[:].rearrange("p (s two) -> p s two", two=2)[:, :, 0],
                    )
                    # One indirect DMA per neighbor column: the HW DGE pairs
                    # ONE offset with each per-partition descriptor (walrus
                    # indirect loads are 2-D only), so a [128, 512] dest with
                    # [128, 1] offsets is the unit of gather. (An inline
                    # CCE-add variant was measured slower: the accumulate
                    # path raises the per-instruction DGE cost ~35%.)
                    x_t = xp.tile([P, S * FEAT], f32)
                    for j in range(S):
                        nc.gpsimd.indirect_dma_start(
                            out=x_t[:, j * FEAT : (j + 1) * FEAT],
                            out_offset=None,
                            in_=feat[:],
                            in_offset=bass.IndirectOffsetOnAxis(
                                ap=off_t[:, j : j + 1], axis=0
                            ),
                        )

                    # tree-sum the 10 rows per node: 10 -> 5 -> (4->2->1) + leftover
                    y5 = sump.tile([P, 5 * FEAT], f32)
                    xv = x_t[:].rearrange("p (s two f) -> p s two f", two=2, f=FEAT)
                    nc.vector.tensor_add(
                        y5[:].rearrange("p (s f) -> p s f", f=FEAT),
                        xv[:, :, 0, :],
                        xv[:, :, 1, :],
                    )
                    y2 = sump.tile([P, 2 * FEAT], f32)
                    y4 = y5[:, 0 : 4 * FEAT].rearrange(
                        "p (s two f) -> p s two f", two=2, f=FEAT
                    )
                    nc.vector.tensor_add(
                        y2[:].rearrange("p (s f) -> p s f", f=FEAT),
                        y4[:, :, 0, :],
                        y4[:, :, 1, :],
                    )
                    agg = aggp.tile([P, FEAT], f32)
                    nc.vector.tensor_add(agg[:], y2[:, :FEAT], y2[:, FEAT:])
                    nc.vector.tensor_add(
                        agg[:], agg[:], y5[:, 4 * FEAT : 5 * FEAT]
                    )

                    # transpose agg -> aggT columns (f onto partitions)
                    ps_t = psT.tile([P, 4 * P], f32)
                    for c in range(4):
                        nc.tensor.transpose(
                            out=ps_t[:, c * P : (c + 1) * P],
                            in_=agg[:, c * P : (c + 1) * P],
                            identity=ident[:],
                        )
                    nc.vector.tensor_copy(
                        aggT[:].rearrange("p (c n) -> p c n", n=Q)[
                            :, :, t * P : (t + 1) * P
                        ],
                        ps_t[:].rearrange("p (c n) -> p c n", n=P),
                    )

                # matmul + leaky relu + store, per embed-half
                for h in range(2):
                    ps_o = psO.tile([P, Q], f32)
                    for c in range(4):
                        nc.tensor.matmul(
                            out=ps_o[:],
                            lhsT=wt_sb[c][:, h * P : (h + 1) * P],
                            rhs=aggT[:, c * Q : (c + 1) * Q],
                            start=(c == 0),
                            stop=(c == 3),
                        )
                    tmp = outp.tile([P, Q], f32)
                    o_sb = outp.tile([P, Q], f32)
                    nc.vector.tensor_scalar_mul(tmp[:], ps_o[:], RRELU_SLOPE)
                    nc.vector.tensor_max(o_sb[:], ps_o[:], tmp[:])
                    nc.sync.dma_start(
                        out=out[h * P : (h + 1) * P, blk * Q : (blk + 1) * Q],
                        in_=o_sb[:],
                    )
    nc.compile()
    return nc


def _prep_inputs(features, weight, neigh_idx):
    feats = np.ascontiguousarray(np.asarray(features), dtype=np.float32)
    w = np.ascontiguousarray(np.asarray(weight), dtype=np.float32)
    idx = np.asarray(neigh_idx)
    assert feats.shape == (N_NODES, FEAT) and w.shape == (EMBED, FEAT)
    assert idx.shape == (BATCH, S)
    idx32 = (
        np.ascontiguousarray(idx.astype(np.int64))
        .view(np.int32)
        .reshape(BATCH, 2 * S)
    )
    return feats, w, idx32


def kernel(features, weight, neigh_idx):
    from concourse.bass_utils import run_bass_kernel_spmd

    if "nc" not in _CACHE:
        _CACHE["nc"] = build_program()
    nc = _CACHE["nc"]

    feats, w, idx32 = _prep_inputs(features, weight, neigh_idx)
    in_maps = [
        {
            "features": feats,
            "weight": w,
            "idx32": idx32[c * B_CORE : (c + 1) * B_CORE],
        }
        for c in range(N_CORES)
    ]
    res = run_bass_kernel_spmd(nc, in_maps, list(range(N_CORES)))
    return np.concatenate(
        [res.results[c]["out"] for c in range(N_CORES)], axis=1
    ).astype(np.float32)


# revision 26
# speedup vs baseline: 1.0299x; 1.0166x over previous
"""GraphSAGE-style encoder kernel for Trainium2 (Bass/Tile), 8-core SPMD.

Computation (see reference):
    agg = features[neigh_idx].mean(axis=1)          # [B, F]
    y   = weight @ agg.T                            # [E, B]
    out = where(y >= 0, y, y * 11/48)               # RReLU eval mode

Sharding: data-parallel over the batch. Core c handles neigh_idx rows
[c*2048, (c+1)*2048); features + weight are replicated. Each core:
  - indirect-DMA-gathers its 20480 neighbor rows (2KB each) from HBM,
    128 nodes x 10 neighbors per gather instruction,
  - tree-sums the 10 rows per node on DVE,
  - transposes agg tiles on the tensor engine (f onto partitions),
  - matmuls with (weight/10)^T accumulated over 4 f-chunks into PSUM,
  - applies leaky-relu as max(y, slope*y) and DMAs out [128e, 512b] tiles.
The per-core 42 MB gather is the roofline term (memory regime).
"""

import numpy as np

N_CORES = 8
N_NODES, FEAT = 100000, 512
EMBED = 256
BATCH, S = 16384, 10
B_CORE = BATCH // N_CORES          # 2048 batch rows per core
P = 128                            # partitions
TILES = B_CORE // P                # 16 node-tiles per core
TPB = 4                            # node-tiles per output block
BLOCKS = TILES // TPB              # 4 blocks -> psum free dim 512
Q = TPB * P                        # 512 nodes per block
RRELU_SLOPE = (1.0 / 8.0 + 1.0 / 3.0) / 2.0   # 11/48

_CACHE = {}


def build_program():
    import concourse.bass as bass
    import concourse.bacc as bacc
    import concourse.mybir as mybir
    import concourse.tile as tile
    from concourse.masks import make_identity

    f32 = mybir.dt.float32
    i32 = mybir.dt.int32

    # Bacc (not plain Bass): its compile() runs the wait-legalization passes
    # (move_matmul_waits_to_ldweights / generate_event_semaphores) that
    # enforce the 1-wait-per-instruction HW constraint walrus checks.
    nc = bacc.Bacc(trn_type="TRN2", num_devices=N_CORES, enable_asserts=False)

    feat = nc.dram_tensor("features", [N_NODES, FEAT], f32, kind="ExternalInput").ap()
    wts = nc.dram_tensor("weight", [EMBED, FEAT], f32, kind="ExternalInput").ap()
    # int64 neigh_idx rows viewed as int32 pairs (little-endian: even cols
    # hold the values, odd cols are zero).
    idx = nc.dram_tensor("idx32", [B_CORE, 2 * S], i32, kind="ExternalInput").ap()
    out = nc.dram_tensor("out", [EMBED, B_CORE], f32, kind="ExternalOutput").ap()

    with tile.TileContext(nc) as tc:
        with (
            tc.tile_pool(name="const", bufs=1) as const,
            tc.tile_pool(name="xp", bufs=3) as xp,
            tc.tile_pool(name="idxp", bufs=4) as idxp,
            tc.tile_pool(name="sump", bufs=2) as sump,
            tc.tile_pool(name="aggp", bufs=2) as aggp,
            tc.tile_pool(name="aggTp", bufs=2) as aggTp,
            tc.tile_pool(name="outp", bufs=3) as outp,
            tc.tile_pool(name="psT", bufs=2, space="PSUM") as psT,
            tc.tile_pool(name="psO", bufs=2, space="PSUM") as psO,
        ):
            ident = const.tile([P, P], f32)
            make_identity(nc, ident[:])

            # --- one-time weight prep: scale by 1/10 (folds the neighbor
            # mean; leaky-relu is positively homogeneous) and transpose to
            # [f, e] chunks for the matmul lhsT.
            w_sb = const.tile([P, 2 * FEAT], f32)      # cols [h*FEAT..] = W rows h*128..
            nc.sync.dma_start(out=w_sb[:, :FEAT], in_=wts[0:P, :])
            nc.sync.dma_start(out=w_sb[:, FEAT:], in_=wts[P : 2 * P, :])
            ws_sb = const.tile([P, 2 * FEAT], f32)
            # one scale op per W-half: walrus allows a single sync wait on
            # TensorScalar instructions, and the two loads complete on
            # different DMA semaphore lanes.
            nc.vector.tensor_scalar_mul(ws_sb[:, :FEAT], w_sb[:, :FEAT], 1.0 / S)
            nc.vector.tensor_scalar_mul(ws_sb[:, FEAT:], w_sb[:, FEAT:], 1.0 / S)

            wt_sb = [
                const.tile([P, 2 * P], f32, name=f"wt{c}", tag=f"wt{c}")
                for c in range(4)
            ]
            for h in range(2):
                wt_ps = psT.tile([P, 4 * P], f32)
                for c in range(4):
                    nc.tensor.transpose(
                        out=wt_ps[:, c * P : (c + 1) * P],
                        in_=ws_sb[:, h * FEAT + c * P : h * FEAT + (c + 1) * P],
                        identity=ident[:],
                    )
                for c in range(4):
                    nc.vector.tensor_copy(
                        wt_sb[c][:, h * P : (h + 1) * P],
                        wt_ps[:, c * P : (c + 1) * P],
                    )

            # --- main loop
            for blk in range(BLOCKS):
                aggT = aggTp.tile([P, 4 * Q], f32)   # [f-part, (chunk c)*(Q nodes)]
                ps_o = [
                    psO.tile([P, Q], f32, name=f"ps_o{h}", tag=f"ps_o{h}")
                    for h in range(2)
                ]
                for t in range(TPB):
                    g = blk * TPB + t
                    idx_t = idxp.tile([P, 2 * S], i32)
                    nc.sync.dma_start(
                        out=idx_t[:], in_=idx[g * P : (g + 1) * P, :]
                    )
                    off_t = idxp.tile([P, S], i32)
                    nc.vector.tensor_copy(
                        off_t[:],
                        idx_t[:].rearrange("p (s two) -> p s two", two=2)[:, :, 0],
                    )
                    # One indirect DMA per neighbor column: the HW DGE pairs
                    # ONE offset with each per-partition descriptor (walrus
                    # indirect loads are 2-D only), so a [128, 512] dest with
                    # [128, 1] offsets is the unit of gather. (An inline
                    # CCE-add variant was measured slower: the accumulate
                    # path raises the per-instruction DGE cost ~35%.)
                    x_t = xp.tile([P, S * FEAT], f32)
                    for j in range(S):
                        nc.gpsimd.indirect_dma_start(
                            out=x_t[:, j * FEAT : (j + 1) * FEAT],
                            out_offset=None,
                            in_=feat[:],
                            in_offset=bass.IndirectOffsetOnAxis(
                                ap=off_t[:, j : j + 1], axis=0
                            ),
                        )

                    # tree-sum the 10 rows per node: 10 -> 5 -> (4->2->1) + leftover
                    y5 = sump.tile([P, 5 * FEAT], f32)
                    xv = x_t[:].rearrange("p (s two f) -> p s two f", two=2, f=FEAT)
                    nc.vector.tensor_add(
                        y5[:].rearrange("p (s f) -> p s f", f=FEAT),
                        xv[:, :, 0, :],
                        xv[:, :, 1, :],
                    )
                    y2 = sump.tile([P, 2 * FEAT], f32)
                    y4 = y5[:, 0 : 4 * FEAT].rearrange(
                        "p (s two f) -> p s two f", two=2, f=FEAT
                    )
                    nc.vector.tensor_add(
                        y2[:].rearrange("p (s f) -> p s f", f=FEAT),
                        y4[:, :, 0, :],
                        y4[:, :, 1, :],
                    )
                    agg = aggp.tile([P, FEAT], f32)
                    nc.vector.tensor_add(agg[:], y2[:, :FEAT], y2[:, FEAT:])
                    nc.vector.tensor_add(
                        agg[:], agg[:], y5[:, 4 * FEAT : 5 * FEAT]
                    )

                    # transpose agg -> aggT columns (f onto partitions)
                    ps_t = psT.tile([P, 4 * P], f32)
                    for c in range(4):
                        nc.tensor.transpose(
                            out=ps_t[:, c * P : (c + 1) * P],
                            in_=agg[:, c * P : (c + 1) * P],
                            identity=ident[:],
                        )
                    nc.vector.tensor_copy(
                        aggT[:].rearrange("p (c n) -> p c n", n=Q)[
                            :, :, t * P : (t + 1) * P
                        ],
                        ps_t[:].rearrange("p (c n) -> p c n", n=P),
                    )

                    # per-tile matmul into this tile's 128-column PSUM slice
                    # (instead of one batched pass after the whole block) so
                    # almost no matmul work remains after the final gather.
                    for h in range(2):
                        for c in range(4):
                            nc.tensor.matmul(
                                out=ps_o[h][:, t * P : (t + 1) * P],
                                lhsT=wt_sb[c][:, h * P : (h + 1) * P],
                                rhs=aggT[:, c * Q + t * P : c * Q + (t + 1) * P],
                                start=(c == 0),
                                stop=(c == 3),
                            )

                # leaky relu + store, per embed-half
                for h in range(2):
                    tmp = outp.tile([P, Q], f32)
                    o_sb = outp.tile([P, Q], f32)
                    nc.vector.tensor_scalar_mul(tmp[:], ps_o[h][:], RRELU_SLOPE)
                    nc.vector.tensor_max(o_sb[:], ps_o[h][:], tmp[:])
                    nc.sync.dma_start(
                        out=out[h * P : (h + 1) * P, blk * Q : (blk + 1) * Q],
                        in_=o_sb[:],
                    )
    nc.compile()
    return nc


def _prep_inputs(features, weight, neigh_idx):
    feats = np.ascontiguousarray(np.asarray(features), dtype=np.float32)
    w = np.ascontiguousarray(np.asarray(weight), dtype=np.float32)
    idx = np.asarray(neigh_idx)
    assert feats.shape == (N_NODES, FEAT) and w.shape == (EMBED, FEAT)
    assert idx.shape == (BATCH, S)
    idx32 = (
        np.ascontiguousarray(idx.astype(np.int64))
        .view(np.int32)
        .reshape(BATCH, 2 * S)
    )
    return feats, w, idx32


def kernel(features, weight, neigh_idx):
    from concourse.bass_utils import run_bass_kernel_spmd

    if "nc" not in _CACHE:
        _CACHE["nc"] = build_program()
    nc = _CACHE["nc"]

    feats, w, idx32 = _prep_inputs(features, weight, neigh_idx)
    in_maps = [
        {
            "features": feats,
            "weight": w,
            "idx32": idx32[c * B_CORE : (c + 1) * B_CORE],
        }
        for c in range(N_CORES)
    ]
    res = run_bass_kernel_spmd(nc, in_maps, list(range(N_CORES)))
    return np.concatenate(
        [res.results[c]["out"] for c in range(N_CORES)], axis=1
    ).astype(np.float32)
